# revision 1
# baseline (speedup 1.0000x reference)
"""Depthwise Conv3D (3x3x3, VALID, stride 1) on 8 Trainium2 NeuronCores.

Strategy: per-channel Toeplitz matmul over the H axis on TensorE.
  out[b,do,ho,wo,f] = sum_{kd,kh,kw} x[b,do+kd,ho+kh,wo+kw,f] * w[kd,kh,kw,f]
For fixed (f,kd,kw) the sum over kh is a banded [H_in=112, HO=110] Toeplitz
matrix applied along H, so one TensorE matmul (contraction over h_in on the
partition dim) handles all 3 kh taps; the 9 (kd,kw) combinations accumulate
in PSUM. Toeplitz matrices are built on the host from the tiny weight tensor.

float32r matmuls run at 1 cycle/row (vs 4 for exact fp32) when the moving
free dim is >=256; fp32r ISA restrictions require a depth-1 moving AP with
an even element count, so the host pre-transposes x into the slab layout
[half, h, f, d*w] making the moving operand a flat 406-element slice
(7 d-planes x 58 w-columns; junk columns at chunk boundaries are never
copied out). W is processed in two halves so the f-complete output staging
fits SBUF; the Toeplitz stream is re-read once per half. Toeplitz and x
are DMA'd in 4-channel batches so every transfer is ~1-2 MB.

Sharding: data-parallel over (batch, D-half) -> 8 shards; weights
(Toeplitz + bias) replicated.
"""

import sys

sys.path.insert(0, "/opt/trn_rl_repo")

from contextlib import ExitStack

import numpy as np

B, D, H, W, F = 4, 16, 112, 112, 64
DO, HO, WO = 14, 110, 110
N_CORES = 8
DO_C = 7  # output d-planes per core
DIN_C = 9  # input d-planes per core
WIN = 58  # input w columns per half
WEV = 56  # output wo columns evacuated per half
W_SPLITS = [0, 54]  # w start of each half (both input and output)
FLAT = DIN_C * WIN  # 522
FLATP = 528  # padded flat (d,w) extent per (h, f)
NMM = DO_C * WIN  # 406: moving-operand length per matmul (even, >=256)
FQ = 4  # channels per DMA batch

MODE = "fp32r"  # "fp32r" (rel err ~2e-4) or "bf16" (faster, rel err ~1e-3)
DBG_SKIP_OUT = False  # ablation: drop output DMAs
DBG_SKIP_MM = False  # ablation: drop matmuls + evacs

_cached = None


def _build(loop_n: int = 1, mode: str | None = None):
    mode = mode or MODE
    from concourse import bacc, mybir, tile

    nc = bacc.Bacc("TRN2", target_bir_lowering=False, debug=False, num_devices=N_CORES)
    f32 = mybir.dt.float32
    mdt = mybir.dt.float32r if mode == "fp32r" else mybir.dt.bfloat16

    # Toeplitz ships as fp16 (10 mantissa bits, ~= fp32r's multiply precision)
    # and is upcast to the matmul dtype on-chip — halves its DMA traffic.
    tdt = mybir.dt.float16 if mode == "fp32r" else mybir.dt.bfloat16
    x_ap = nc.dram_tensor("xp", [2, H, F, FLATP], mdt, kind="ExternalInput").ap()
    t_ap = nc.dram_tensor(
        "toep", [F // FQ, H, FQ, 9, HO], tdt, kind="ExternalInput"
    ).ap()
    b_ap = nc.dram_tensor("biasbc", [128, F], f32, kind="ExternalInput").ap()
    o_ap = nc.dram_tensor("out", [DO_C, HO, WO, F], f32, kind="ExternalOutput").ap()

    with tile.TileContext(nc) as tc, ExitStack() as ctx:
        slab_pool = ctx.enter_context(tc.tile_pool(name="slab", bufs=5))
        toep_pool = ctx.enter_context(tc.tile_pool(name="toep", bufs=2))
        stage_pool = ctx.enter_context(tc.tile_pool(name="stage", bufs=1))
        psum_pool = ctx.enter_context(tc.tile_pool(name="psum", bufs=8, space="PSUM"))
        const_pool = ctx.enter_context(tc.tile_pool(name="const", bufs=1))

        bias_t = const_pool.tile([128, F], f32, name="bias_t")
        nc.sync.dma_start(out=bias_t[:], in_=b_ap[:])

        loop_ctx = tc.For_i(0, loop_n) if loop_n > 1 else None
        if loop_ctx is not None:
            ctx.enter_context(loop_ctx)

        for ih, w0 in enumerate(W_SPLITS):
            stage = stage_pool.tile([HO, DO_C, WEV, F], f32, name="stage", tag="stage")
            for q in range(F // FQ):
                # input DMAs go on the ACT HWDGE ring so they never queue
                # behind the output drain on the SP ring
                toep_h = toep_pool.tile([H, FQ, 9, HO], tdt, name="toep_h", tag="th")
                nc.scalar.dma_start(out=toep_h[:], in_=t_ap[q])
                if mode == "fp32r":
                    toep_q = toep_pool.tile(
                        [H, FQ, 9, HO], mdt, name="toep_q", tag="tq"
                    )
                    if q % 2 == 0:
                        nc.vector.tensor_copy(toep_q[:], toep_h[:])
                    else:
                        nc.scalar.activation(
                            toep_q[:],
                            toep_h[:],
                            mybir.ActivationFunctionType.Copy,
                        )
                else:
                    toep_q = toep_h
                slab_q = slab_pool.tile([H, FQ, FLATP], mdt, name="slab_q", tag="sq")
                nc.scalar.dma_start(out=slab_q[:], in_=x_ap[ih, :, q * FQ : (q + 1) * FQ, :])
                for fi in range(FQ):
                    if DBG_SKIP_MM:
                        break
                    f = q * FQ + fi
                    psum_t = psum_pool.tile(
                        [HO, DO_C, WIN], f32, name="psum_t", tag="ps"
                    )
                    for kd in range(3):
                        for kw in range(3):
                            tap = kd * 3 + kw
                            off = kd * WIN + kw
                            nc.tensor.matmul(
                                psum_t[:],
                                lhsT=toep_q[:, fi, tap, :],
                                rhs=slab_q[:, fi, off : off + NMM],
                                start=(tap == 0),
                                stop=(tap == 8),
                            )
                    # evacuate PSUM -> staging (dropping junk w cols), add bias
                    if f % 2 == 0:
                        nc.vector.tensor_scalar_add(
                            stage[:, :, :, f],
                            psum_t[:, :, 0:WEV],
                            bias_t[0:HO, f : f + 1],
                        )
                    else:
                        nc.scalar.activation(
                            stage[:, :, :, f],
                            psum_t[:, :, 0:WEV],
                            mybir.ActivationFunctionType.Identity,
                            bias=bias_t[0:HO, f : f + 1],
                        )
            for do in range(DO_C):
                if DBG_SKIP_OUT:
                    break
                nc.sync.dma_start(
                    out=o_ap[do, :, w0 : w0 + WEV, :], in_=stage[:, do]
                )

    nc.compile()
    return nc


def _np_dt(mode: str):
    if mode == "fp32r":
        return np.float32
    import ml_dtypes

    return ml_dtypes.bfloat16


def _toeplitz(w: np.ndarray, mode: str | None = None) -> np.ndarray:
    mode = mode or MODE
    t = np.zeros((F, H, 9, HO), np.float32)
    ho = np.arange(HO)
    for kd in range(3):
        for kh in range(3):
            for kw in range(3):
                t[:, ho + kh, kd * 3 + kw, ho] = w[kd, kh, kw, 0, :][:, None]
    # [F, H, 9, HO] -> [F//FQ, H, FQ, 9, HO] quad-batched layout
    t = np.ascontiguousarray(
        t.reshape(F // FQ, FQ, H, 9 * HO).transpose(0, 2, 1, 3)
    ).reshape(F // FQ, H, FQ, 9, HO)
    if mode == "fp32r":
        return t.astype(np.float16)
    return t.astype(_np_dt(mode))


def _pack_x(xs: np.ndarray, mode: str | None = None) -> np.ndarray:
    """[DIN_C, H, W, F] -> [2, H, F, FLATP] slab layout (half, h, f, (d, w))."""
    mode = mode or MODE
    xp = np.zeros((2, H, F, FLATP), _np_dt(mode))
    for ih, w0 in enumerate(W_SPLITS):
        chunk = xs[:, :, w0 : w0 + WIN, :]  # [d, h, w, f]
        xp[ih, :, :, :FLAT] = chunk.transpose(1, 3, 0, 2).reshape(H, F, FLAT)
    return xp


def kernel(x: np.ndarray, w: np.ndarray, b: np.ndarray) -> np.ndarray:
    global _cached
    if _cached is None:
        _cached = _build()
    nc = _cached

    from concourse.bass_utils import run_bass_kernel_spmd

    x = np.asarray(x, np.float32)
    toep = _toeplitz(np.asarray(w, np.float32))
    bias_bc = np.tile(np.asarray(b, np.float32)[None, :], (128, 1))

    in_maps = []
    for core in range(N_CORES):
        bb, dh = divmod(core, 2)
        in_maps.append(
            {
                "xp": _pack_x(x[bb, dh * DO_C : dh * DO_C + DIN_C]),
                "toep": toep,
                "biasbc": bias_bc,
            }
        )

    res = run_bass_kernel_spmd(nc, in_maps, list(range(N_CORES)))

    out = np.empty((B, DO, HO, WO, F), np.float32)
    for core in range(N_CORES):
        bb, dh = divmod(core, 2)
        out[bb, dh * DO_C : (dh + 1) * DO_C] = res.results[core]["out"]
    return out



# revision 2
# speedup vs baseline: 4.7198x; 4.7198x over previous
"""Depthwise Conv3D (3x3x3, VALID, stride 1) on 8 Trainium2 NeuronCores.

Strategy v2: per-channel 2D-banded-Toeplitz matmul on TensorE.

TensorE matmul cost is the number of streamed moving-operand columns
(1 col/cycle for 16-bit dtypes), independent of how many contraction rows
are live. So the goal is to absorb as many kernel taps as possible into
the stationary operand. A 1D Toeplitz over H absorbs only kh (9 re-streams
of x). Here the stationary operand is a 2D-banded Toeplitz over (d, h)
jointly:

  T[(d,h'), (do,ho')] = w[d-do, h'-ho', kw, f]  for d-do,h'-ho' in [0,3)

with contraction rows (d,h') = 9*13 = 117 <= 128 and PSUM partitions
(do,ho') = 7*11 = 77 <= 128. That absorbs kd AND kh, leaving only the
3 kw taps as re-streams: streamed columns drop from vol*9/110 to
vol*3/77 per channel (2.3x less TensorE time). H is processed in 10
blocks of 11 output rows (13 input rows, stride 11), which tile H=112
exactly — no junk columns anywhere.

Everything ships fp16 (PSUM accumulates fp32): x slab 16.9MB, Toeplitz
3.5MB, output 10.8MB per core. Host packs x into the windowed slab
layout and unpacks the blocked output.

Sharding: data-parallel over (batch, D-half) -> 8 shards; weights
(Toeplitz + bias) replicated.
"""

import sys

sys.path.insert(0, "/opt/trn_rl_repo")

from contextlib import ExitStack

import numpy as np

B, D, H, W, F = 4, 16, 112, 112, 64
DO, HO, WO = 14, 110, 110
N_CORES = 8
DO_C = 7  # output d-planes per core
DIN_C = 9  # input d-planes per core
HB = 11  # output h rows per block
NB = 10  # h blocks (11*10 = 110 = HO exactly)
HWIN = 13  # input h rows per block window
ROWS = DIN_C * HWIN  # 117 contraction rows
P = DO_C * HB  # 77 PSUM partitions
FQ = 4  # channels per DMA batch
CHUNKS = [(0, 4), (4, 4), (8, 2)]  # (first block, n blocks) per PSUM tile

_cached = None


def _build(loop_n: int = 1):
    from concourse import bacc, mybir, tile

    nc = bacc.Bacc("TRN2", target_bir_lowering=False, debug=False, num_devices=N_CORES)
    f32 = mybir.dt.float32
    f16 = mybir.dt.float16

    x_ap = nc.dram_tensor("xp", [F // FQ, ROWS, FQ, NB, W], f16, kind="ExternalInput").ap()
    t_ap = nc.dram_tensor("toep", [F // FQ, ROWS, FQ, 3, P], f16, kind="ExternalInput").ap()
    b_ap = nc.dram_tensor("biasbc", [128, F], f32, kind="ExternalInput").ap()
    o_ap = nc.dram_tensor("out", [F // FQ, P, NB, WO, FQ], f16, kind="ExternalOutput").ap()

    with tile.TileContext(nc) as tc, ExitStack() as ctx:
        slab_pool = ctx.enter_context(tc.tile_pool(name="slab", bufs=4))
        toep_pool = ctx.enter_context(tc.tile_pool(name="toep", bufs=2))
        stage_pool = ctx.enter_context(tc.tile_pool(name="stage", bufs=2))
        psum_pool = ctx.enter_context(tc.tile_pool(name="psum", bufs=8, space="PSUM"))
        const_pool = ctx.enter_context(tc.tile_pool(name="const", bufs=1))

        bias_t = const_pool.tile([128, F], f32, name="bias_t")
        nc.sync.dma_start(out=bias_t[:], in_=b_ap[:])

        loop_ctx = tc.For_i(0, loop_n) if loop_n > 1 else None
        if loop_ctx is not None:
            ctx.enter_context(loop_ctx)

        for q in range(F // FQ):
            # input DMAs on the ACT HWDGE ring so they never queue behind
            # the output drain on the SP ring
            toep_t = toep_pool.tile([ROWS, FQ, 3, P], f16, name="toep_t", tag="th")
            nc.scalar.dma_start(out=toep_t[:], in_=t_ap[q])
            slab_q = slab_pool.tile([ROWS, FQ, NB, W], f16, name="slab_q", tag="sq")
            nc.scalar.dma_start(out=slab_q[:], in_=x_ap[q])
            stage = stage_pool.tile([P, NB, WO, FQ], f16, name="stage", tag="stage")
            for fi in range(FQ):
                f = q * FQ + fi
                psums = [
                    psum_pool.tile([P, nb, WO], f32, name="psum_t", tag="ps")
                    for (_, nb) in CHUNKS
                ]
                for kw in range(3):
                    for ci, (c0, nb) in enumerate(CHUNKS):
                        nc.tensor.matmul(
                            psums[ci][:],
                            lhsT=toep_t[:, fi, kw, :],
                            rhs=slab_q[:, fi, c0 : c0 + nb, kw : kw + WO],
                            start=(kw == 0),
                            stop=(kw == 2),
                        )
                # evacuate PSUM -> staging (fp32 -> fp16), add bias
                for ci, (c0, nb) in enumerate(CHUNKS):
                    if f % 2 == 0:
                        nc.vector.tensor_scalar_add(
                            stage[:, c0 : c0 + nb, :, fi],
                            psums[ci][:],
                            bias_t[0:P, f : f + 1],
                        )
                    else:
                        nc.scalar.activation(
                            stage[:, c0 : c0 + nb, :, fi],
                            psums[ci][:],
                            mybir.ActivationFunctionType.Identity,
                            bias=bias_t[0:P, f : f + 1],
                        )
            nc.sync.dma_start(out=o_ap[q], in_=stage[:])

    nc.compile()
    return nc


def _toeplitz(w: np.ndarray) -> np.ndarray:
    """w [3,3,3,1,F] -> [F//FQ, ROWS, FQ, 3, P] 2D-banded Toeplitz, fp16."""
    t = np.zeros((F, DIN_C, HWIN, 3, DO_C, HB), np.float32)
    hp = np.arange(HB)
    for kd in range(3):
        for kh in range(3):
            for kw in range(3):
                for do in range(DO_C):
                    t[:, do + kd, hp + kh, kw, do, hp] = w[kd, kh, kw, 0, :][:, None]
    t = t.reshape(F // FQ, FQ, ROWS, 3, P).transpose(0, 2, 1, 3, 4)
    return np.ascontiguousarray(t).astype(np.float16)


def _pack_x(xs: np.ndarray) -> np.ndarray:
    """[DIN_C, H, W, F] -> [F//FQ, ROWS, FQ, NB, W] windowed slab, fp16."""
    xs16 = np.ascontiguousarray(xs).astype(np.float16)
    sd, sh, sw, sf = xs16.strides
    xw = np.lib.stride_tricks.as_strided(
        xs16, shape=(DIN_C, NB, HWIN, W, F), strides=(sd, HB * sh, sh, sw, sf)
    )
    # -> [f, d, h', hb, w] -> [q, (d,h'), fi, hb, w]
    xp = xw.transpose(4, 0, 2, 1, 3).reshape(F // FQ, FQ, ROWS, NB, W)
    xp = xp.transpose(0, 2, 1, 3, 4)
    return np.ascontiguousarray(xp)


def _unpack_out(o: np.ndarray) -> np.ndarray:
    """[F//FQ, P, NB, WO, FQ] fp16 -> [DO_C, HO, WO, F] fp32."""
    o = o.reshape(F // FQ, DO_C, HB, NB, WO, FQ).transpose(1, 3, 2, 4, 0, 5)
    return o.reshape(DO_C, HO, WO, F).astype(np.float32)


def kernel(x: np.ndarray, w: np.ndarray, b: np.ndarray) -> np.ndarray:
    global _cached
    if _cached is None:
        _cached = _build()
    nc = _cached

    from concourse.bass_utils import run_bass_kernel_spmd

    x = np.asarray(x, np.float32)
    toep = _toeplitz(np.asarray(w, np.float32))
    bias_bc = np.tile(np.asarray(b, np.float32)[None, :], (128, 1))

    in_maps = []
    for core in range(N_CORES):
        bb, dh = divmod(core, 2)
        in_maps.append(
            {
                "xp": _pack_x(x[bb, dh * DO_C : dh * DO_C + DIN_C]),
                "toep": toep,
                "biasbc": bias_bc,
            }
        )

    res = run_bass_kernel_spmd(nc, in_maps, list(range(N_CORES)))

    out = np.empty((B, DO, HO, WO, F), np.float32)
    for core in range(N_CORES):
        bb, dh = divmod(core, 2)
        out[bb, dh * DO_C : (dh + 1) * DO_C] = _unpack_out(res.results[core]["out"])
    return out


# revision 9
# speedup vs baseline: 5.0773x; 1.0757x over previous
"""Depthwise Conv3D (3x3x3, VALID, stride 1) on 8 Trainium2 NeuronCores.

Strategy v2.1: per-channel 2D-banded-Toeplitz matmul on TensorE.

TensorE matmul cost is the number of streamed moving-operand columns
(1 col/cycle for 16-bit dtypes), independent of how many contraction rows
are live. So the goal is to absorb as many kernel taps as possible into
the stationary operand. A 1D Toeplitz over H absorbs only kh (9 re-streams
of x). Here the stationary operand is a 2D-banded Toeplitz over (d, h)
jointly:

  T[(d,h'), (do,ho')] = w[d-do, h'-ho', kw, f]  for d-do,h'-ho' in [0,3)

with contraction rows (d,h') = 9*13 = 117 <= 128 and PSUM partitions
(do,ho') = 7*11 = 77 <= 128. That absorbs kd AND kh, leaving only the
3 kw taps as re-streams: streamed columns drop from vol*9/110 to
vol*3/77 per channel (2.3x less TensorE time). H is processed in 10
blocks of 11 output rows (13 input rows, stride 11), which tile H=112
exactly — no junk columns anywhere.

Everything ships fp16 (PSUM accumulates fp32): x slab 16.9MB, Toeplitz
3.5MB, output 10.8MB per core. DMA instruction count is kept low (the
per-DMA fixed cost is ~0.7us): slab and output move in 2-channel-quad
batches, the Toeplitz in one transfer; HBM layouts are partition-major
so every batched transfer is contiguous per partition. PSUM evacuation
(bias add + fp32->fp16) is spread over DVE, GpSimd and ACT so no single
engine chains behind the matmuls.

Sharding: data-parallel over (batch, D-half) -> 8 shards; weights
(Toeplitz + bias) replicated.
"""

import sys

sys.path.insert(0, "/opt/trn_rl_repo")

from contextlib import ExitStack

import numpy as np

B, D, H, W, F = 4, 16, 112, 112, 64
DO, HO, WO = 14, 110, 110
N_CORES = 8
DO_C = 7  # output d-planes per core
DIN_C = 9  # input d-planes per core
HB = 11  # output h rows per block
NB = 10  # h blocks (11*10 = 110 = HO exactly)
HWIN = 13  # input h rows per block window
ROWS = DIN_C * HWIN  # 117 contraction rows
P = DO_C * HB  # 77 PSUM partitions
FQ = 4  # channels per matmul group
QB = 2  # channel-quads per DMA batch
NQ = F // FQ  # 16 quads
CHUNKS = [(0, 4), (4, 4), (8, 2)]  # (first block, n blocks) per PSUM tile

import os

DBG_SKIP_MM = bool(int(os.environ.get("DBG_SKIP_MM", "0")))  # drop matmuls + evacs
DBG_SKIP_OUT = bool(int(os.environ.get("DBG_SKIP_OUT", "0")))  # drop output DMAs
DBG_SKIP_IN = bool(int(os.environ.get("DBG_SKIP_IN", "0")))  # drop input slab DMAs

_cached = None


def _build(loop_n: int = 1):
    from concourse import bacc, mybir, tile

    nc = bacc.Bacc("TRN2", target_bir_lowering=False, debug=False, num_devices=N_CORES)
    f32 = mybir.dt.float32
    f16 = mybir.dt.float16

    x_ap = nc.dram_tensor("xp", [ROWS, NQ, FQ, NB, W], f16, kind="ExternalInput").ap()
    t_ap = nc.dram_tensor("toep", [ROWS, NQ, FQ, 3, P], f16, kind="ExternalInput").ap()
    b_ap = nc.dram_tensor("biasbc", [128, F], f32, kind="ExternalInput").ap()
    o_ap = nc.dram_tensor("out", [P, NQ, NB, WO, FQ], f16, kind="ExternalOutput").ap()

    with tile.TileContext(nc) as tc, ExitStack() as ctx:
        slab_pool = ctx.enter_context(tc.tile_pool(name="slab", bufs=3))
        toep_pool = ctx.enter_context(tc.tile_pool(name="toep", bufs=2))
        stage_pool = ctx.enter_context(tc.tile_pool(name="stage", bufs=2))
        psum_pool = ctx.enter_context(tc.tile_pool(name="psum", bufs=8, space="PSUM"))
        const_pool = ctx.enter_context(tc.tile_pool(name="const", bufs=1))

        bias_t = const_pool.tile([128, F], f32, name="bias_t")
        nc.sync.dma_start(out=bias_t[:], in_=b_ap[:])

        loop_ctx = tc.For_i(0, loop_n) if loop_n > 1 else None
        if loop_ctx is not None:
            ctx.enter_context(loop_ctx)

        # whole Toeplitz in one transfer per iteration, on the SP ring
        toep_t = toep_pool.tile([ROWS, NQ, FQ, 3, P], f16, name="toep_t", tag="th")
        nc.sync.dma_start(out=toep_t[:], in_=t_ap[:])

        evac_engines = [None, None, None]
        ei = 0
        for qb in range(NQ // QB):
            # slab batch of QB quads on the ACT HWDGE ring
            slab_b = slab_pool.tile([ROWS, QB, FQ, NB, W], f16, name="slab_b", tag="sq")
            if not DBG_SKIP_IN:
                nc.scalar.dma_start(
                    out=slab_b[:], in_=x_ap[:, qb * QB : (qb + 1) * QB]
                )
            stage = stage_pool.tile([P, QB, NB, WO, FQ], f16, name="stage", tag="st")
            for qi in range(QB):
                q = qb * QB + qi
                for fi in range(FQ):
                    if DBG_SKIP_MM:
                        break
                    f = q * FQ + fi
                    psums = [
                        psum_pool.tile([P, nb, WO], f32, name="psum_t", tag="ps")
                        for (_, nb) in CHUNKS
                    ]
                    for kw in range(3):
                        for ci, (c0, nb) in enumerate(CHUNKS):
                            nc.tensor.matmul(
                                psums[ci][:],
                                lhsT=toep_t[:, q, fi, kw, :],
                                rhs=slab_b[:, qi, fi, c0 : c0 + nb, kw : kw + WO],
                                start=(kw == 0),
                                stop=(kw == 2),
                            )
                    # evacuate PSUM -> staging (fp32 -> fp16), add bias;
                    # alternate DVE / ACT (GpSimd cannot read PSUM on TRN2)
                    for ci, (c0, nb) in enumerate(CHUNKS):
                        eng = ei % 2
                        ei += 1
                        if eng == 0:
                            nc.vector.tensor_scalar_add(
                                stage[:, qi, c0 : c0 + nb, :, fi],
                                psums[ci][:],
                                bias_t[0:P, f : f + 1],
                            )
                        else:
                            nc.scalar.activation(
                                stage[:, qi, c0 : c0 + nb, :, fi],
                                psums[ci][:],
                                mybir.ActivationFunctionType.Identity,
                                bias=bias_t[0:P, f : f + 1],
                            )
            if not DBG_SKIP_OUT:
                if DBG_SKIP_MM:
                    nc.sync.dma_start(
                        out=o_ap[:, qb * QB : (qb + 1) * QB],
                        in_=slab_b[0:P, :, :, :, 0:110],
                    )
                else:
                    nc.sync.dma_start(
                        out=o_ap[:, qb * QB : (qb + 1) * QB], in_=stage[:]
                    )

    nc.compile()
    return nc


def _toeplitz(w: np.ndarray) -> np.ndarray:
    """w [3,3,3,1,F] -> [ROWS, NQ, FQ, 3, P] 2D-banded Toeplitz, fp16."""
    t = np.zeros((F, DIN_C, HWIN, 3, DO_C, HB), np.float32)
    hp = np.arange(HB)
    for kd in range(3):
        for kh in range(3):
            for kw in range(3):
                for do in range(DO_C):
                    t[:, do + kd, hp + kh, kw, do, hp] = w[kd, kh, kw, 0, :][:, None]
    # [F, ROWS, 3, P] -> [ROWS, NQ, FQ, 3, P]
    t = t.reshape(F, ROWS, 3, P).transpose(1, 0, 2, 3).reshape(ROWS, NQ, FQ, 3, P)
    return np.ascontiguousarray(t).astype(np.float16)


def _pack_x(xs: np.ndarray) -> np.ndarray:
    """[DIN_C, H, W, F] -> [ROWS, NQ, FQ, NB, W] windowed slab, fp16."""
    xs16 = np.ascontiguousarray(xs).astype(np.float16)
    sd, sh, sw, sf = xs16.strides
    xw = np.lib.stride_tricks.as_strided(
        xs16, shape=(DIN_C, NB, HWIN, W, F), strides=(sd, HB * sh, sh, sw, sf)
    )
    # [d, hb, h', w, f] -> [(d,h'), f, hb, w] -> [ROWS, NQ, FQ, NB, W]
    xp = xw.transpose(0, 2, 4, 1, 3).reshape(ROWS, F, NB, W)
    return np.ascontiguousarray(xp).reshape(ROWS, NQ, FQ, NB, W)


def _unpack_out(o: np.ndarray) -> np.ndarray:
    """[P, NQ, NB, WO, FQ] fp16 -> [DO_C, HO, WO, F] fp32."""
    o = o.reshape(DO_C, HB, NQ, NB, WO, FQ).transpose(0, 3, 1, 4, 2, 5)
    return o.reshape(DO_C, HO, WO, F).astype(np.float32)


def kernel(x: np.ndarray, w: np.ndarray, b: np.ndarray) -> np.ndarray:
    global _cached
    if _cached is None:
        _cached = _build()
    nc = _cached

    from concourse.bass_utils import run_bass_kernel_spmd

    x = np.asarray(x, np.float32)
    toep = _toeplitz(np.asarray(w, np.float32))
    bias_bc = np.tile(np.asarray(b, np.float32)[None, :], (128, 1))

    in_maps = []
    for core in range(N_CORES):
        bb, dh = divmod(core, 2)
        in_maps.append(
            {
                "xp": _pack_x(x[bb, dh * DO_C : dh * DO_C + DIN_C]),
                "toep": toep,
                "biasbc": bias_bc,
            }
        )

    res = run_bass_kernel_spmd(nc, in_maps, list(range(N_CORES)))

    out = np.empty((B, DO, HO, WO, F), np.float32)
    for core in range(N_CORES):
        bb, dh = divmod(core, 2)
        out[bb, dh * DO_C : (dh + 1) * DO_C] = _unpack_out(res.results[core]["out"])
    return out
